# revision 21
# baseline (speedup 1.0000x reference)
"""HGT layer kernel for 8 Trainium2 NeuronCores (v2, batched dma_gather).

Strategy: nodes are relabeled so node types are contiguous (padded to 128);
edges are sorted by destination and packed into per-128-dst-node tiles of
NS_t slabs x 128 slots.  Each core owns a contiguous range of dst tiles, so
segment softmax and aggregation are core-local (no collectives).

v2 replaces the per-slab indirect DMAs + one-hot q-select of the baseline:
  - kv rows (k||v, 256B fp8e3) are fetched with batched `dma_gather`
    (<=1024 int16 idxs per call).  The 50304-row table exceeds int16 range,
    so each tile's slots are split into a low-table region (src < 32768)
    and a high-table region at a statically known slab boundary; per-core
    count variation inside a region is padded with junk-valid idxs whose
    slot codes are inert.
  - per-edge q rows (W_Q W_e^T mu/sqrt(hd) folded, bf16 256B) are fetched
    with dma_gather from qt_tab[et*ownN + dst_local] (5*6400 rows < 32767),
    eliminating the edge-type one-hot build and 5 select-matmuls per slab.
  - all per-slab vector ops are batched into per-tile ops; softmax
    denominators ride the aggregation matmul (cols 128:136).
  - FFN runs in transposed layout (p1T = W1_chunk^T-free matmuls against
    h1T) so only 2 PE transposes per tile; FFN weights in bf16; gelu is a
    single scalar-engine Gelu activation per chunk.
Gathers round-robin the 4 SWDGE queues (num_swdge_queues=4): multi-queue
descriptor emission is ~2.8x faster than a single queue on HW.  Compiled
with Bacc (inserts the gpsimd `mlp` library load required by
InstDMAGatherAnt; plain Bass + generate_event_semaphores crashes the Q7).
Known limits found on this build: dma_gather <= 1024 idxs per call (larger
crashes the exec unit), idx tables int16, elem_size % 256B == 0.
"""

import math
import numpy as np

import concourse.mybir as mybir
import concourse.bass as bass
import concourse.bacc as bacc
from concourse.tile import TileContext
from concourse.masks import make_identity
from concourse.bass_utils import run_bass_kernel_spmd

N_NODES = 50000
N_EDGES = 800000
D = 128
H = 8
HD = 16
NT = 3
NET = 5
LN_EPS = 1e-5
NCORES = 8
P = 128
LOW = 32768          # low-table rows (int16 idx range)
CH = 8               # max slabs per dma_gather (1024 idxs)
F32 = mybir.dt.float32
BF16 = mybir.dt.bfloat16
F16 = mybir.dt.float16
F8 = mybir.dt.float8e3
I16 = mybir.dt.int16


def _bc_ap(ap, ins_pos, n):
    """Insert a stride-0 axis of size n at position ins_pos of an AP."""
    dims = list(ap.ap)
    dims.insert(ins_pos, [0, n])
    return bass.AP(tensor=ap.tensor, offset=ap.offset, ap=dims)


def _chunks(n):
    """Split n slabs into chunks of <= CH slabs."""
    out = []
    c0 = 0
    while c0 < n:
        c1 = min(n, c0 + CH)
        out.append((c0, c1))
        c0 = c1
    return out


def _build(nc, chunk_types, n_pad, T, SA, SB, has_ln_gb1,
           has_ln_gb2, has_bout, has_b1, has_b2):
    ownN = T * P
    NS = [a + b for a, b in zip(SA, SB)]
    NSm = max(NS)
    xt_bf = nc.dram_tensor("xt_bf", [n_pad // P, D, P], BF16,
                           kind="ExternalInput")
    xt_own = nc.dram_tensor("xt_own", [T, D, P], BF16, kind="ExternalInput")
    x_own = nc.dram_tensor("x_own", [ownN, D], F32, kind="ExternalInput")
    wkv = nc.dram_tensor("wkv", [NT, D, 2 * D], BF16, kind="ExternalInput")
    wqe_own = nc.dram_tensor("wqe_own", [T, D, NET * D], BF16,
                             kind="ExternalInput")
    wout = nc.dram_tensor("wout", [D, D], BF16, kind="ExternalInput")
    w1 = nc.dram_tensor("w1", [D, 4 * D], BF16, kind="ExternalInput")
    w2 = nc.dram_tensor("w2", [4 * D, D], BF16, kind="ExternalInput")
    kvidx = nc.dram_tensor("kvidx", [T, P, NSm * 8], I16, kind="ExternalInput")
    qidx = nc.dram_tensor("qidx", [T, P, NSm * 8], I16, kind="ExternalInput")
    codes = nc.dram_tensor("codes", [T, P, NSm], F16, kind="ExternalInput")
    iota_f = nc.dram_tensor("iota_f", [P, P], F16, kind="ExternalInput")
    gb = nc.dram_tensor("gb", [6, P, 4 * D], F32, kind="ExternalInput")
    out = nc.dram_tensor("out", [ownN, D], F32, kind="ExternalOutput")

    kv_tab = nc.dram_tensor("kv_tab", [n_pad, 2 * D], F8)
    qt_tab = nc.dram_tensor("qt_tab", [NET * ownN, D], BF16)

    with TileContext(nc) as tc:
        with (
            tc.tile_pool(name="const", bufs=1) as const,
            tc.tile_pool(name="wpool", bufs=1) as wpool,
        ):
            ident = const.tile([P, P], F32)
            make_identity(nc, ident)
            iota_sb = const.tile([P, P], F16)
            nc.sync.dma_start(out=iota_sb, in_=iota_f[:, :])
            eps_sb = const.tile([P, 1], F32)
            nc.vector.memset(eps_sb, LN_EPS)
            wkv_sb = wpool.tile([P, NT, 2 * D], BF16)
            nc.sync.dma_start(out=wkv_sb, in_=wkv.rearrange("t d e -> d t e"))
            wout_sb = wpool.tile([P, D], BF16)
            nc.sync.dma_start(out=wout_sb, in_=wout[:, :])
            w1_sb = wpool.tile([P, 4 * D], BF16)
            nc.sync.dma_start(out=w1_sb, in_=w1[:, :])
            w2_sb = wpool.tile([P, 4, D], BF16)
            nc.sync.dma_start(out=w2_sb, in_=w2.rearrange("(c p) d -> p c d", p=P))
            gb_sb = wpool.tile([P, 6, 4 * D], F32)
            nc.sync.dma_start(out=gb_sb, in_=gb.rearrange("g p d -> p g d"))

            # ---------------- prologue: build kv/qt tables ----------------
            # pro stays open through the tile phase: its SBUF is never
            # reused, so tile-phase pools carry no WAR deps on the prologue
            # and the gathers can start the moment the tables are written.
            with tc.tile_pool(name="pro", bufs=4) as pro:
              with tc.tile_pool(name="pro_ps", bufs=2, space="PSUM") as pro_ps:
                # qt first (its tail overlaps the gather phase), then kv
                # (which gates the gathers).
                QB = 4
                for i0 in range(0, T, QB):
                    nb = min(QB, T - i0)
                    xf = pro.tile([P, QB, P], BF16, tag="xfo")
                    nc.sync.dma_start(
                        out=xf[:, 0:nb, :],
                        in_=xt_own[i0:i0 + nb].rearrange("c d p -> d c p"))
                    wq_sb = pro.tile([P, QB, NET * D], BF16, tag="wq")
                    nc.sync.dma_start(
                        out=wq_sb[:, 0:nb, :],
                        in_=wqe_own[i0:i0 + nb].rearrange("c d e -> d c e"))
                    qt_sb = pro.tile([P, QB, NET * D], BF16, tag="qts")
                    for i in range(nb):
                        qt1_ps = pro_ps.tile([P, 4 * D], F32, tag="q1")
                        nc.tensor.matmul(out=qt1_ps, lhsT=xf[:, i, :],
                                         rhs=wq_sb[:, i, 0:4 * D],
                                         start=True, stop=True)
                        qt2_ps = pro_ps.tile([P, D], F32, tag="q2")
                        nc.tensor.matmul(out=qt2_ps, lhsT=xf[:, i, :],
                                         rhs=wq_sb[:, i, 4 * D:NET * D],
                                         start=True, stop=True)
                        if i % 2 == 0:
                            nc.vector.tensor_copy(out=qt_sb[:, i, 0:4 * D],
                                                  in_=qt1_ps)
                            nc.vector.tensor_copy(out=qt_sb[:, i, 4 * D:],
                                                  in_=qt2_ps)
                        else:
                            nc.scalar.copy(out=qt_sb[:, i, 0:4 * D],
                                           in_=qt1_ps)
                            nc.scalar.copy(out=qt_sb[:, i, 4 * D:],
                                           in_=qt2_ps)
                    for e in range(NET):
                        weng = nc.sync if e % 2 == 0 else nc.scalar
                        weng.dma_start(
                            out=qt_tab[e * ownN + i0 * P:
                                       e * ownN + (i0 + nb) * P, :].rearrange(
                                "(i p) f -> p i f", p=P),
                            in_=qt_sb[:, 0:nb, e * D:(e + 1) * D])
            # ---------------- edge tiles + fused FFN ----------------
              with (
                  tc.tile_pool(name="idx", bufs=3) as idxp,
                  tc.tile_pool(name="gat", bufs=3) as gat,
                  tc.tile_pool(name="edge", bufs=2) as edge,
                  tc.tile_pool(name="small", bufs=3) as small,
                  tc.tile_pool(name="ffn", bufs=2) as ffn,
                  tc.tile_pool(name="agg_ps", bufs=1, space="PSUM") as agg_psp,
                  tc.tile_pool(name="ffn_ps", bufs=1, space="PSUM") as ffn_ps,
                  tc.tile_pool(name="p1_ps", bufs=1, space="PSUM") as p1_psp,
                  tc.tile_pool(name="qpre", bufs=1) as qprep,
                  tc.tile_pool(name="kv_ps", bufs=2, space="PSUM") as kvpsp,
              ):
                  qn = [0]

                  def _q():
                      qn[0] = (qn[0] + 1) % 4
                      return qn[0]

                  # ---- q prefetch for the first M tiles: these only need
                  # qt_tab, so they fill the gpsimd queue while the kv table
                  # is still being built (a blocked kv gather would stall the
                  # in-order queue).
                  M = min(6, T)
                  qpre = {}
                  for t in range(M):
                      ns = NS[t]
                      qidx_sb = idxp.tile([P, NSm * 8], I16, tag="qi")
                      nc.sync.dma_start(out=qidx_sb[:, 0:ns * 8],
                                        in_=qidx[t, :, 0:ns * 8])
                      q_g = qprep.tile([P, NSm, D], BF16, tag=f"qp{t}")
                      for (c0, c1) in _chunks(ns):
                          n = (c1 - c0) * P
                          nc.gpsimd.dma_gather(
                              q_g[:, c0:c1, :], qt_tab[:, :],
                              qidx_sb[:, c0 * 8:c1 * 8], n, n, D,
                              queue_num=_q())
                      qpre[t] = q_g

                  # kv table build (overlaps the prefetched q gathers)
                  KB = 2
                  for c0 in range(0, len(chunk_types), KB):
                      nb = min(KB, len(chunk_types) - c0)
                      xf = pro.tile([P, KB, P], BF16, tag="xft")
                      nc.sync.dma_start(
                          out=xf[:, 0:nb, :],
                          in_=xt_bf[c0:c0 + nb].rearrange("c d p -> d c p"))
                      kv_ps = kvpsp.tile([P, KB, 2 * D], F32, tag="kv")
                      for i in range(nb):
                          nc.tensor.matmul(
                              out=kv_ps[:, i, :], lhsT=xf[:, i, :],
                              rhs=wkv_sb[:, chunk_types[c0 + i], :],
                              start=True, stop=True)
                      kv_sb = pro.tile([P, KB, 2 * D], F8, tag="kvs")
                      if (c0 // KB) % 2 == 0:
                          nc.vector.tensor_copy(out=kv_sb[:, 0:nb, :],
                                                in_=kv_ps[:, 0:nb, :])
                      else:
                          nc.scalar.copy(out=kv_sb[:, 0:nb, :],
                                         in_=kv_ps[:, 0:nb, :])
                      weng = nc.sync if (c0 // KB) % 2 == 0 else nc.scalar
                      weng.dma_start(
                          out=kv_tab[c0 * P:(c0 + nb) * P, :].rearrange(
                              "(c p) e -> p c e", p=P),
                          in_=kv_sb[:, 0:nb, :])

                  for t in range(T):
                      ns, sa, sb_ = NS[t], SA[t], SB[t]
                      kvidx_sb = idxp.tile([P, NSm * 8], I16, tag="ki")
                      nc.sync.dma_start(out=kvidx_sb[:, 0:ns * 8],
                                        in_=kvidx[t, :, 0:ns * 8])
                      codes_sb = idxp.tile([P, NSm], F16, tag="co")
                      nc.sync.dma_start(out=codes_sb[:, 0:ns],
                                        in_=codes[t, :, 0:ns])

                      if t < M:
                          q_g = qpre[t]
                      else:
                          qidx_sb = idxp.tile([P, NSm * 8], I16, tag="qi")
                          nc.sync.dma_start(out=qidx_sb[:, 0:ns * 8],
                                            in_=qidx[t, :, 0:ns * 8])
                          q_g = gat.tile([P, NSm, D], BF16, tag="qg")
                          for (c0, c1) in _chunks(ns):
                              n = (c1 - c0) * P
                              nc.gpsimd.dma_gather(
                                  q_g[:, c0:c1, :], qt_tab[:, :],
                                  qidx_sb[:, c0 * 8:c1 * 8], n, n, D,
                                  queue_num=_q())
                      kv_g = gat.tile([P, NSm, 2 * D], F8, tag="kg")
                      for (c0, c1) in _chunks(sa):
                          n = (c1 - c0) * P
                          nc.gpsimd.dma_gather(
                              kv_g[:, c0:c1, :], kv_tab[0:LOW, :],
                              kvidx_sb[:, c0 * 8:c1 * 8], n, n, 2 * D,
                              queue_num=_q())
                      for (c0, c1) in _chunks(sb_):
                          n = (c1 - c0) * P
                          nc.gpsimd.dma_gather(
                              kv_g[:, sa + c0:sa + c1, :], kv_tab[LOW:n_pad, :],
                              kvidx_sb[:, (sa + c0) * 8:(sa + c1) * 8],
                              n, n, 2 * D, queue_num=_q())

                      # one-hot (dst-code): mt[p, s, j]
                      mt = edge.tile([P, NSm, P], BF16, tag="mt")
                      nc.vector.tensor_tensor(
                          out=mt[:, 0:ns, :],
                          in0=codes_sb[:, 0:ns].to_broadcast([P, ns, P]),
                          in1=_bc_ap(iota_sb[:, :], 1, ns),
                          op=mybir.AluOpType.is_equal)
                      # scores
                      prod = edge.tile([P, NSm, D], BF16, tag="pr")
                      nc.vector.tensor_mul(out=prod[:, 0:ns, :],
                                           in0=q_g[:, 0:ns, :],
                                           in1=kv_g[:, 0:ns, 0:D])
                      s_sb = small.tile([P, NSm * H], F32, tag="s")
                      nc.vector.reduce_sum(
                          out=s_sb[:, 0:ns * H],
                          in_=prod[:, 0:ns, :].rearrange(
                              "p s (h f) -> p (s h) f", h=H),
                          axis=mybir.AxisListType.X)
                      ex = small.tile([P, NSm * H], BF16, tag="ex")
                      nc.scalar.activation(
                          out=ex[:, 0:ns * H], in_=s_sb[:, 0:ns * H],
                          func=mybir.ActivationFunctionType.Exp)
                      rhs = edge.tile([P, NSm, P + H], BF16, tag="rhs")
                      nc.vector.tensor_tensor(
                          out=rhs[:, 0:ns, 0:P].rearrange(
                              "p s (h f) -> p s h f", h=H),
                          in0=kv_g[:, 0:ns, D:2 * D].rearrange(
                              "p s (h f) -> p s h f", h=H),
                          in1=ex[:, 0:ns * H].rearrange(
                              "p (s h) -> p s h", h=H).to_broadcast(
                              [P, ns, H, HD]),
                          op=mybir.AluOpType.mult)
                      nc.scalar.copy(
                          out=rhs[:, 0:ns, P:P + H],
                          in_=ex[:, 0:ns * H].rearrange("p (s h) -> p s h", h=H))

                      agg_ps = agg_psp.tile([P, P + H], F32)
                      for b in range(ns):
                          nc.tensor.matmul(out=agg_ps, lhsT=mt[:, b, :],
                                           rhs=rhs[:, b, :],
                                           start=(b == 0), stop=(b == ns - 1))

                      den = small.tile([P, H], F32, tag="den")
                      nc.vector.tensor_scalar_add(out=den, in0=agg_ps[:, P:P + H],
                                                  scalar1=1e-10)
                      rcp = small.tile([P, H], F32, tag="rcp")
                      nc.vector.reciprocal(out=rcp, in_=den)
                      aggn = ffn.tile([P, D], F32, tag="aggn")
                      nc.vector.tensor_tensor(
                          out=aggn[:, :].rearrange("p (h f) -> p h f", h=H),
                          in0=agg_ps[:, 0:P].rearrange("p (h f) -> p h f", h=H),
                          in1=rcp[:, :].to_broadcast([P, H, HD]),
                          op=mybir.AluOpType.mult)

                      # ---- W_out + residual + LN1 ----
                      aggn_f_ps = ffn_ps.tile([P, P], F32, tag="tp")
                      nc.tensor.transpose(out=aggn_f_ps, in_=aggn, identity=ident)
                      aggn_f = ffn.tile([P, P], BF16, tag="aggnf")
                      nc.scalar.copy(out=aggn_f, in_=aggn_f_ps)
                      mh_ps = ffn_ps.tile([P, D], F32, tag="mh")
                      nc.tensor.matmul(out=mh_ps, lhsT=aggn_f, rhs=wout_sb,
                                       start=True, stop=True)
                      x_sb = ffn.tile([P, D], F32, tag="xo")
                      nc.sync.dma_start(out=x_sb, in_=x_own[t * P:(t + 1) * P, :])
                      h1p = ffn.tile([P, D], F32, tag="h1p")
                      nc.vector.tensor_add(out=h1p, in0=x_sb, in1=mh_ps)
                      if has_bout:
                          nc.vector.tensor_add(out=h1p, in0=h1p, in1=gb_sb[:, 4, 0:D])
                      h1 = _layer_norm(nc, small, ffn, h1p, eps_sb, gb_sb, 0, 1,
                                       has_ln_gb1, "h1")
                      # ---- FFN (transposed layout) ----
                      h1f_ps = ffn_ps.tile([P, P], F32, tag="tp")
                      nc.tensor.transpose(out=h1f_ps, in_=h1, identity=ident)
                      h1f = ffn.tile([P, P], BF16, tag="h1f")
                      nc.scalar.copy(out=h1f, in_=h1f_ps)
                      p2_ps = ffn_ps.tile([P, D], F32, tag="p2")
                      for cc in range(4):
                          p1_ps = p1_psp.tile([P, P], F32, tag=f"p1{cc % 2}")
                          nc.tensor.matmul(
                              out=p1_ps, lhsT=w1_sb[:, cc * P:(cc + 1) * P],
                              rhs=h1f, start=True, stop=True)
                          gf = ffn.tile([P, P], BF16, tag=f"gf{cc % 2}")
                          if has_b1:
                              nc.scalar.activation(
                                  out=gf, in_=p1_ps,
                                  func=mybir.ActivationFunctionType.Gelu,
                                  bias=gb_sb[:, 5, cc:cc + 1])
                          else:
                              nc.scalar.activation(
                                  out=gf, in_=p1_ps,
                                  func=mybir.ActivationFunctionType.Gelu)
                          nc.tensor.matmul(out=p2_ps, lhsT=gf,
                                           rhs=w2_sb[:, cc, :],
                                           start=(cc == 0), stop=(cc == 3))
                      o1 = ffn.tile([P, D], F32, tag="o1")
                      nc.vector.tensor_add(out=o1, in0=h1, in1=p2_ps)
                      if has_b2:
                          nc.vector.tensor_add(out=o1, in0=o1, in1=gb_sb[:, 4, D:2 * D])
                      o2 = _layer_norm(nc, small, ffn, o1, eps_sb, gb_sb, 2, 3,
                                       has_ln_gb2, "o2")
                      nc.sync.dma_start(out=out[t * P:(t + 1) * P, :], in_=o2)
    return nc


def _layer_norm(nc, small, ffn, xin, eps_sb, gb_sb, gi, bi, has_gb, tag):
    stats = small.tile([P, 6], F32, tag=tag + "st")
    nc.vector.bn_stats(out=stats, in_=xin)
    mv = small.tile([P, 2], F32, tag=tag + "mv")
    nc.vector.bn_aggr(out=mv, in_=stats)
    sd = small.tile([P, 1], F32, tag=tag + "sd")
    nc.scalar.activation(out=sd, in_=mv[:, 1:2],
                         func=mybir.ActivationFunctionType.Sqrt,
                         bias=eps_sb)
    rs = small.tile([P, 1], F32, tag=tag + "rs")
    nc.vector.reciprocal(out=rs, in_=sd)
    nmb = small.tile([P, 1], F32, tag=tag + "nm")
    nc.vector.tensor_mul(out=nmb, in0=mv[:, 0:1], in1=rs)
    nc.vector.tensor_scalar_mul(out=nmb, in0=nmb, scalar1=-1.0)
    h = ffn.tile([P, D], F32, tag=tag + "h")
    nc.scalar.activation(out=h, in_=xin,
                         func=mybir.ActivationFunctionType.Identity,
                         bias=nmb, scale=rs)
    if has_gb:
        nc.vector.tensor_mul(out=h, in0=h, in1=gb_sb[:, gi, 0:D])
        nc.vector.tensor_add(out=h, in0=h, in1=gb_sb[:, bi, 0:D])
    return h


_CACHE = {}


def _wrap_idx(flat):
    """[S*128] slot-ordered idxs -> [128, S*8] wrapped int16 layout."""
    n = flat.shape[0]
    a = flat.reshape(n // 16, 16).T.astype(np.int16)   # [16, n//16]
    return np.tile(a, (8, 1))


def kernel(x, edge_index, edge_type, node_type,
           W_Q, W_K, W_V, W_edge, mu,
           W_out, b_out, ln1_g, ln1_b, W1, b1, W2, b2, ln2_g, ln2_b):
    x = np.asarray(x, np.float32)
    src = np.asarray(edge_index[0], np.int64)
    dst = np.asarray(edge_index[1], np.int64)
    et = np.asarray(edge_type, np.int64)
    nt = np.asarray(node_type, np.int64)
    N = x.shape[0]
    E = src.shape[0]

    # ---- node relabeling: group by type, pad each group to 128; within
    # each group, deal nodes round-robin by in-degree so per-tile edge
    # counts are balanced ----
    import heapq
    indeg = np.bincount(dst, minlength=N)
    new_id = np.zeros(N, np.int64)
    base = 0
    chunk_types = []
    for t in range(NT):
        origs = np.where(nt == t)[0]
        ntiles = int(math.ceil(len(origs) / P))
        order_t = origs[np.argsort(-indeg[origs], kind="stable")]
        heap = [(0, k) for k in range(ntiles)]
        fill = np.zeros(ntiles, np.int64)
        for node in order_t:
            load, k = heapq.heappop(heap)
            new_id[node] = base + k * P + fill[k]
            fill[k] += 1
            if fill[k] < P:
                heapq.heappush(heap, (load + int(indeg[node]), k))
        chunk_types += [t] * ntiles
        base += ntiles * P
    n_pad = base
    n_tiles = n_pad // P
    T = int(math.ceil(n_tiles / NCORES))
    n_grid = T * NCORES * P
    ownN = T * P

    srcN = new_id[src]
    dstN = new_id[dst]

    order = np.argsort(dstN, kind="stable")
    ds, ss, es = dstN[order], srcN[order], et[order]
    tile_id = ds // P
    counts = np.bincount(tile_id, minlength=T * NCORES)
    starts = np.concatenate([[0], np.cumsum(counts)])

    # per (core, tile-slot): split edges into low-src / high-src
    lowmask = ss < LOW
    nA = np.zeros((NCORES, T), np.int64)
    nB = np.zeros((NCORES, T), np.int64)
    for g in range(T * NCORES):
        c, tt = g // T, g % T
        m = lowmask[starts[g]:starts[g + 1]]
        nA[c, tt] = int(m.sum())
        nB[c, tt] = int((~m).sum())
    SA = [int(math.ceil(nA[:, tt].max() / P)) for tt in range(T)]
    SB = [int(math.ceil(max(1, nB[:, tt].max()) / P)) for tt in range(T)]
    NS = [a + b for a, b in zip(SA, SB)]
    NSm = max(NS)

    kvidx = np.zeros((NCORES, T, P, NSm * 8), np.int16)
    qidx = np.zeros((NCORES, T, P, NSm * 8), np.int16)
    codes = np.full((NCORES, T, P, NSm), 255.0, np.float16)
    for g in range(T * NCORES):
        c, tt = g // T, g % T
        sl = slice(starts[g], starts[g + 1])
        sse, dse, ete = ss[sl], ds[sl], es[sl]
        m = sse < LOW
        sa, sb_, ns = SA[tt], SB[tt], NS[tt]
        nslots = ns * P
        kvf = np.zeros(nslots, np.int64)
        qf = np.zeros(nslots, np.int64)
        cof = np.full(nslots, 255.0, np.float16)
        na = int(m.sum())
        nb_ = int((~m).sum())
        # region A: low-src edges at slots [0, na)
        kvf[0:na] = sse[m]
        qf[0:na] = ete[m] * ownN + (dse[m] - c * ownN)
        cof[0:na] = (dse[m] % P).astype(np.float16)
        # region B: high-src edges at slots [sa*P, sa*P+nb)
        kvf[sa * P:sa * P + nb_] = sse[~m] - LOW
        qf[sa * P:sa * P + nb_] = ete[~m] * ownN + (dse[~m] - c * ownN)
        cof[sa * P:sa * P + nb_] = (dse[~m] % P).astype(np.float16)
        kvidx[c, tt, :, 0:ns * 8] = _wrap_idx(kvf)
        qidx[c, tt, :, 0:ns * 8] = _wrap_idx(qf)
        codes[c, tt, :, 0:ns] = cof.reshape(ns, P).T

    import ml_dtypes
    x_perm = np.zeros((n_grid, D), np.float32)
    x_perm[new_id] = x
    xt_grid = np.ascontiguousarray(
        x_perm.reshape(n_grid // P, P, D).transpose(0, 2, 1)
    ).astype(ml_dtypes.bfloat16)

    # ---- fold mu and 1/sqrt(hd) into combined Q-side weights ----
    W_Q = np.asarray(W_Q, np.float32)
    W_K = np.asarray(W_K, np.float32)
    W_V = np.asarray(W_V, np.float32)
    W_edge = np.asarray(W_edge, np.float32)
    mu = np.asarray(mu, np.float32)
    wkv = np.zeros((NT, D, 2 * D), np.float32)
    wqe = np.zeros((NT, D, NET * D), np.float32)
    for t in range(NT):
        for h in range(H):
            sl = slice(h * HD, (h + 1) * HD)
            wkv[t, sl, sl] = W_K[t, h]
            wkv[t, sl, D + h * HD:D + (h + 1) * HD] = W_V[t, h]
            for e in range(NET):
                comb = (W_Q[t, h] @ W_edge[e, h].T) * (mu[h, e] / math.sqrt(HD))
                wqe[t, sl, e * D + h * HD:e * D + (h + 1) * HD] = comb
    wkv_bf = wkv.astype(ml_dtypes.bfloat16)
    wqe_bf = wqe.astype(ml_dtypes.bfloat16)
    ct = np.asarray(chunk_types + [0] * (n_grid // P - len(chunk_types)),
                    np.int64)
    wqe_own_all = wqe_bf[ct]  # [n_grid//P, D, NET*D]

    gb = np.zeros((6, P, 4 * D), np.float32)
    gb[0, :, :D] = ln1_g
    gb[1, :, :D] = ln1_b
    gb[2, :, :D] = ln2_g
    gb[3, :, :D] = ln2_b
    gb[4, :, :D] = b_out
    gb[4, :, D:2 * D] = b2
    # b1 transposed for the transposed-FFN layout: gb[5][p, c] = b1[c*128+p]
    gb[5, :, 0:4] = np.asarray(b1, np.float32).reshape(4, P).T
    iota_f = np.tile(np.arange(P, dtype=np.float16)[None, :], (P, 1))

    has_ln_gb1 = bool(np.any(ln1_g != 1) or np.any(ln1_b != 0))
    has_ln_gb2 = bool(np.any(ln2_g != 1) or np.any(ln2_b != 0))
    has_bout = bool(np.any(b_out != 0))
    has_b1 = bool(np.any(b1 != 0))
    has_b2 = bool(np.any(b2 != 0))

    key = (n_pad, T, tuple(SA), tuple(SB), has_ln_gb1, has_ln_gb2,
           has_bout, has_b1, has_b2, tuple(chunk_types))
    if key not in _CACHE:
        nc = bacc.Bacc("TRN2", num_swdge_queues=4)
        _build(nc, chunk_types, n_pad, T, SA, SB,
               has_ln_gb1, has_ln_gb2, has_bout, has_b1, has_b2)
        nc.compile()
        _CACHE[key] = nc
    nc = _CACHE[key]

    xt_tab = np.ascontiguousarray(xt_grid[:n_pad // P])
    in_maps = []
    for c in range(NCORES):
        t0, t1 = c * T, (c + 1) * T
        in_maps.append({
            "xt_bf": xt_tab,
            "xt_own": xt_grid[t0:t1],
            "x_own": x_perm[t0 * P:t1 * P],
            "wkv": wkv_bf,
            "wqe_own": wqe_own_all[t0:t1],
            "wout": np.asarray(W_out, np.float32).astype(ml_dtypes.bfloat16),
            "w1": np.asarray(W1, np.float32).astype(ml_dtypes.bfloat16),
            "w2": np.asarray(W2, np.float32).astype(ml_dtypes.bfloat16),
            "kvidx": kvidx[c],
            "qidx": qidx[c],
            "codes": codes[c],
            "iota_f": iota_f, "gb": gb,
        })
    import os
    trace = bool(os.environ.get("KBENCH_TRACE"))
    res = run_bass_kernel_spmd(nc, in_maps, core_ids=list(range(NCORES)),
                               trace=trace)
    global LAST_RESULT
    LAST_RESULT = res
    out_new = np.concatenate([r["out"] for r in res.results], axis=0)
    return out_new[new_id].astype(np.float32)


LAST_RESULT = None
